# revision 4
# baseline (speedup 1.0000x reference)
"""Trainium2 kernel for nn_Add_Square_2654289789550 (Square-Attack patch loop).

Key reduction: each of the 5000 steps adds +/-2*EPS to a square patch and then
clamps every pixel to [max(x-EPS,0), min(x+EPS,1)] -- an interval whose width
is <= 2*EPS.  Since the step magnitude (2*EPS) always covers the interval, a
touched pixel saturates to exactly `lo` or `hi` depending only on the sign of
the LAST patch that touched it; the init also places every pixel exactly at
lo/hi.  All randomness comes from the fixed key jax.random.key(1) -- fully
input-independent -- so the per-pixel final sign map S is a compile-time
constant (computed host-side with a bit-exact numpy port of jax's threefry
PRNG).  The device kernel then computes

    out = clip(x + EPS * S, 0, 1)

which matches the reference to <=2 ulp (verified: rel err 1.4e-8 against a
full f32 simulation of the 5000-step loop).

Distribution: x is flattened (150528 elems = 8 * 128 * 147) and split
contiguously across the 8 NeuronCores.  Per core (raw bass, no Tile):
x-load on the SP HWDGE ring and delta-load on the ACT ring in parallel,
DVE does one tensor_add + one fused two-scalar clamp, out-store on SP.
"""

import sys

import numpy as np

sys.path.insert(0, "/opt/trn_rl_repo")

N_CORES = 8
C, H = 3, 224
EPS32 = np.float32(0.05)
N_QUERIES = 5000
P_INIT = 0.8
P, F = 128, 147  # per-core tile: 128 partitions x 147 f32


# ---------------------------------------------------------------------------
# Pure-numpy, bit-exact port of jax.random's threefry2x32 PRNG
# ---------------------------------------------------------------------------

_ROT = ((13, 15, 26, 6), (17, 29, 16, 24))


def _rotl(x, r):
    return (x << np.uint32(r)) | (x >> np.uint32(32 - r))


def _threefry2x32(key, x0, x1):
    with np.errstate(over="ignore"):
        ks0 = np.asarray(key[0], np.uint32)
        ks1 = np.asarray(key[1], np.uint32)
        ks2 = ks0 ^ ks1 ^ np.uint32(0x1BD11BDA)
        x0 = x0.astype(np.uint32) + ks0
        x1 = x1.astype(np.uint32) + ks1
        ks = (ks0, ks1, ks2)
        for i in range(5):
            for r in _ROT[i % 2]:
                x0 = x0 + x1
                x1 = _rotl(x1, r)
                x1 = x1 ^ x0
            x0 = x0 + ks[(i + 1) % 3]
            x1 = x1 + ks[(i + 2) % 3] + np.uint32(i + 1)
        return x0, x1


def _threefry_counts(key, counts):
    counts = counts.ravel().astype(np.uint32)
    n = counts.size
    if n % 2:
        counts = np.concatenate([counts, np.zeros(1, np.uint32)])
    half = counts.size // 2
    y0, y1 = _threefry2x32(key, counts[:half], counts[half:])
    return np.concatenate([y0, y1])[:n]


def _fold_in(key, data):
    hi = np.uint32((int(data) >> 32) & 0xFFFFFFFF)
    lo = np.uint32(int(data) & 0xFFFFFFFF)
    y0, y1 = _threefry2x32((key[0], key[1]), np.array([hi]), np.array([lo]))
    return np.array([y0[0], y1[0]], dtype=np.uint32)


def _split(key):
    out = _threefry_counts((key[0], key[1]), np.arange(4, dtype=np.uint32))
    return out[:2], out[2:]


def _uniform(key, n):
    bits = _threefry_counts((key[0], key[1]), np.arange(n, dtype=np.uint32))
    return _bits_to_uniform(bits)


def _bits_to_uniform(bits):
    f = ((bits >> np.uint32(9)) | np.uint32(0x3F800000)).view(np.float32)
    return np.maximum(np.float32(0.0), f - np.float32(1.0))


# ---------------------------------------------------------------------------
# Reference schedule + final sign map (input-independent constant)
# ---------------------------------------------------------------------------


def _p_selection(it):
    if 10 < it <= 50:
        return P_INIT / 2
    elif 50 < it <= 200:
        return P_INIT / 4
    elif 200 < it <= 500:
        return P_INIT / 8
    elif 500 < it <= 1000:
        return P_INIT / 16
    elif 1000 < it <= 2000:
        return P_INIT / 32
    elif 2000 < it <= 4000:
        return P_INIT / 64
    elif 4000 < it <= 6000:
        return P_INIT / 128
    elif 6000 < it <= 8000:
        return P_INIT / 256
    elif 8000 < it:
        return P_INIT / 512
    return P_INIT


def _s_schedule():
    import math

    n_features = C * H * H
    return np.array(
        [
            max(int(round(math.sqrt(_p_selection(i) * n_features / C))), 1)
            for i in range(N_QUERIES)
        ],
        dtype=np.int32,
    )


def _sign_map():
    N = N_QUERIES
    base = np.array([0, 1], dtype=np.uint32)  # jax.random.key(1)
    k0 = _fold_in(base, 0)
    init_sign = np.sign(
        np.float32(2.0) * _uniform(k0, C * H) - np.float32(1.0)
    ).reshape(C, 1, H)

    # batched fold_in(base, t+1): cipher(key=base, (0, t+1))
    ts = np.arange(1, N + 1, dtype=np.uint32)
    f0, f1 = _threefry2x32((base[0], base[1]), np.zeros(N, np.uint32), ts)
    # batched split: counts [0,1,2,3] -> pairs (0,2) and (1,3);
    # k1 = (y0 of both pairs), k2 = (y1 of both pairs)
    y0a, y1a = _threefry2x32((f0, f1), np.zeros(N, np.uint32), np.full(N, 2, np.uint32))
    y0b, y1b = _threefry2x32((f0, f1), np.ones(N, np.uint32), np.full(N, 3, np.uint32))
    # u = uniform(k1, ()): counts [0] padded to (0,0)
    ub, _ = _threefry2x32((y0a, y0b), np.zeros(N, np.uint32), np.zeros(N, np.uint32))
    us = _bits_to_uniform(ub)
    # sgn = uniform(k2, (3,)): counts [0,1,2] padded to pairs (0,2),(1,0)
    sa0, sa1 = _threefry2x32((y1a, y1b), np.zeros(N, np.uint32), np.full(N, 2, np.uint32))
    sb0, _ = _threefry2x32((y1a, y1b), np.ones(N, np.uint32), np.zeros(N, np.uint32))
    sg = np.sign(
        np.float32(2.0) * _bits_to_uniform(np.stack([sa0, sb0, sa1], axis=1))
        - np.float32(1.0)
    )

    s_arr = _s_schedule()
    vh = np.floor(us * (np.float32(H) - s_arr.astype(np.float32))).astype(np.int32)
    S = np.broadcast_to(init_sign, (C, H, H)).astype(np.float32).copy()
    for t in range(N):
        v, s = int(vh[t]), int(s_arr[t])
        S[:, v : v + s, v : v + s] = sg[t][:, None, None]
    return S


# ---------------------------------------------------------------------------
# Bass kernel: per-core out = clip(x + d, 0, 1) on a [128,147] f32 tile
# ---------------------------------------------------------------------------

_NC = None
_DELTA = None
LAST_EXEC_NS = None
PROFILE = False


def _build_nc():
    import concourse.bass as bass
    import concourse.mybir as mybir

    f32 = mybir.dt.float32
    nc = bass.Bass("TRN2", num_devices=N_CORES)
    x_d = nc.declare_dram_parameter("x", [P, F], f32, isOutput=False)
    d_d = nc.declare_dram_parameter("d", [P, F], f32, isOutput=False)
    o_d = nc.declare_dram_parameter("out", [P, F], f32, isOutput=True)
    xt = nc.alloc_sbuf_tensor("xt_sb", [P, F], f32)
    dt = nc.alloc_sbuf_tensor("dt_sb", [P, F], f32)

    with (
        nc.Block(no_gpsimd_drain=True) as block,
        nc.semaphore("dma_sem") as dma_sem,
        nc.semaphore("v_sem") as v_sem,
    ):
        # x on the SP HWDGE ring, d on the ACT ring: the loads run in parallel
        @block.sync
        def _(sync):
            sync.dma_start(out=xt[:], in_=x_d[:]).then_inc(dma_sem, 16)
            sync.wait_ge(v_sem, 1)
            sync.dma_start(out=o_d[:], in_=xt[:]).then_inc(dma_sem, 16)
            sync.wait_ge(dma_sem, 48)

        @block.scalar
        def _(scalar):
            scalar.dma_start(out=dt[:], in_=d_d[:]).then_inc(dma_sem, 16)

        @block.vector
        def _(vector):
            vector.wait_ge(dma_sem, 32)
            vector.tensor_add(out=xt[:], in0=xt[:], in1=dt[:])
            vector.tensor_scalar(
                out=xt[:],
                in0=xt[:],
                scalar1=0.0,
                scalar2=1.0,
                op0=mybir.AluOpType.max,
                op1=mybir.AluOpType.min,
            ).then_inc(v_sem, 1)

    return nc


def kernel(x):
    global _NC, _DELTA, LAST_EXEC_NS
    from concourse.bass_utils import run_bass_kernel_spmd

    if _NC is None:
        _NC = _build_nc()
    if _DELTA is None:
        _DELTA = (EPS32 * _sign_map()).astype(np.float32).reshape(N_CORES, P, F)

    x_np = np.asarray(x)
    in_dtype = x_np.dtype
    x_np = np.ascontiguousarray(x_np, dtype=np.float32).reshape(N_CORES, P, F)
    in_maps = [
        {"x": np.ascontiguousarray(x_np[i]), "d": _DELTA[i]} for i in range(N_CORES)
    ]
    res = run_bass_kernel_spmd(
        _NC, in_maps, core_ids=list(range(N_CORES)), trace=PROFILE
    )
    LAST_EXEC_NS = res.exec_time_ns
    out = np.stack([res.results[i]["out"] for i in range(N_CORES)])
    return out.reshape(1, C, H, H).astype(in_dtype)


# revision 6
# speedup vs baseline: 1.0085x; 1.0085x over previous
"""Trainium2 kernel for nn_Add_Square_2654289789550 (Square-Attack patch loop).

Key reduction: each of the 5000 steps adds +/-2*EPS to a square patch and then
clamps every pixel to [max(x-EPS,0), min(x+EPS,1)] -- an interval whose width
is <= 2*EPS.  Since the step magnitude (2*EPS) always covers the interval, a
touched pixel saturates to exactly `lo` or `hi` depending only on the sign of
the LAST patch that touched it; the init also places every pixel exactly at
lo/hi.  All randomness comes from the fixed key jax.random.key(1) -- fully
input-independent -- so the per-pixel final sign map S is a compile-time
constant (computed host-side with a bit-exact numpy port of jax's threefry
PRNG).  The device kernel then computes

    out = clip(x + EPS * S, 0, 1)

which matches the reference to <=2 ulp (verified: rel err 1.4e-8 against a
full f32 simulation of the 5000-step loop).

Distribution: x is flattened (150528 elems = 8 * 128 * 147) and split
contiguously across the 8 NeuronCores.  Per core (raw bass, no Tile):
x-load on the SP HWDGE ring and delta-load on the ACT ring in parallel,
DVE does one tensor_add + one fused two-scalar clamp, out-store on SP.
"""

import sys

import numpy as np

sys.path.insert(0, "/opt/trn_rl_repo")

N_CORES = 8
C, H = 3, 224
EPS32 = np.float32(0.05)
N_QUERIES = 5000
P_INIT = 0.8
P, F = 128, 147  # per-core tile: 128 partitions x 147 f32


# ---------------------------------------------------------------------------
# Pure-numpy, bit-exact port of jax.random's threefry2x32 PRNG
# ---------------------------------------------------------------------------

_ROT = ((13, 15, 26, 6), (17, 29, 16, 24))


def _rotl(x, r):
    return (x << np.uint32(r)) | (x >> np.uint32(32 - r))


def _threefry2x32(key, x0, x1):
    with np.errstate(over="ignore"):
        ks0 = np.asarray(key[0], np.uint32)
        ks1 = np.asarray(key[1], np.uint32)
        ks2 = ks0 ^ ks1 ^ np.uint32(0x1BD11BDA)
        x0 = x0.astype(np.uint32) + ks0
        x1 = x1.astype(np.uint32) + ks1
        ks = (ks0, ks1, ks2)
        for i in range(5):
            for r in _ROT[i % 2]:
                x0 = x0 + x1
                x1 = _rotl(x1, r)
                x1 = x1 ^ x0
            x0 = x0 + ks[(i + 1) % 3]
            x1 = x1 + ks[(i + 2) % 3] + np.uint32(i + 1)
        return x0, x1


def _threefry_counts(key, counts):
    counts = counts.ravel().astype(np.uint32)
    n = counts.size
    if n % 2:
        counts = np.concatenate([counts, np.zeros(1, np.uint32)])
    half = counts.size // 2
    y0, y1 = _threefry2x32(key, counts[:half], counts[half:])
    return np.concatenate([y0, y1])[:n]


def _fold_in(key, data):
    hi = np.uint32((int(data) >> 32) & 0xFFFFFFFF)
    lo = np.uint32(int(data) & 0xFFFFFFFF)
    y0, y1 = _threefry2x32((key[0], key[1]), np.array([hi]), np.array([lo]))
    return np.array([y0[0], y1[0]], dtype=np.uint32)


def _split(key):
    out = _threefry_counts((key[0], key[1]), np.arange(4, dtype=np.uint32))
    return out[:2], out[2:]


def _uniform(key, n):
    bits = _threefry_counts((key[0], key[1]), np.arange(n, dtype=np.uint32))
    return _bits_to_uniform(bits)


def _bits_to_uniform(bits):
    f = ((bits >> np.uint32(9)) | np.uint32(0x3F800000)).view(np.float32)
    return np.maximum(np.float32(0.0), f - np.float32(1.0))


# ---------------------------------------------------------------------------
# Reference schedule + final sign map (input-independent constant)
# ---------------------------------------------------------------------------


def _p_selection(it):
    if 10 < it <= 50:
        return P_INIT / 2
    elif 50 < it <= 200:
        return P_INIT / 4
    elif 200 < it <= 500:
        return P_INIT / 8
    elif 500 < it <= 1000:
        return P_INIT / 16
    elif 1000 < it <= 2000:
        return P_INIT / 32
    elif 2000 < it <= 4000:
        return P_INIT / 64
    elif 4000 < it <= 6000:
        return P_INIT / 128
    elif 6000 < it <= 8000:
        return P_INIT / 256
    elif 8000 < it:
        return P_INIT / 512
    return P_INIT


def _s_schedule():
    import math

    n_features = C * H * H
    return np.array(
        [
            max(int(round(math.sqrt(_p_selection(i) * n_features / C))), 1)
            for i in range(N_QUERIES)
        ],
        dtype=np.int32,
    )


def _sign_map():
    N = N_QUERIES
    base = np.array([0, 1], dtype=np.uint32)  # jax.random.key(1)
    k0 = _fold_in(base, 0)
    init_sign = np.sign(
        np.float32(2.0) * _uniform(k0, C * H) - np.float32(1.0)
    ).reshape(C, 1, H)

    # batched fold_in(base, t+1): cipher(key=base, (0, t+1))
    ts = np.arange(1, N + 1, dtype=np.uint32)
    f0, f1 = _threefry2x32((base[0], base[1]), np.zeros(N, np.uint32), ts)
    # batched split: counts [0,1,2,3] -> pairs (0,2) and (1,3);
    # k1 = (y0 of both pairs), k2 = (y1 of both pairs)
    y0a, y1a = _threefry2x32((f0, f1), np.zeros(N, np.uint32), np.full(N, 2, np.uint32))
    y0b, y1b = _threefry2x32((f0, f1), np.ones(N, np.uint32), np.full(N, 3, np.uint32))
    # u = uniform(k1, ()): counts [0] padded to (0,0)
    ub, _ = _threefry2x32((y0a, y0b), np.zeros(N, np.uint32), np.zeros(N, np.uint32))
    us = _bits_to_uniform(ub)
    # sgn = uniform(k2, (3,)): counts [0,1,2] padded to pairs (0,2),(1,0)
    sa0, sa1 = _threefry2x32((y1a, y1b), np.zeros(N, np.uint32), np.full(N, 2, np.uint32))
    sb0, _ = _threefry2x32((y1a, y1b), np.ones(N, np.uint32), np.zeros(N, np.uint32))
    sg = np.sign(
        np.float32(2.0) * _bits_to_uniform(np.stack([sa0, sb0, sa1], axis=1))
        - np.float32(1.0)
    )

    s_arr = _s_schedule()
    vh = np.floor(us * (np.float32(H) - s_arr.astype(np.float32))).astype(np.int32)
    S = np.broadcast_to(init_sign, (C, H, H)).astype(np.float32).copy()
    for t in range(N):
        v, s = int(vh[t]), int(s_arr[t])
        S[:, v : v + s, v : v + s] = sg[t][:, None, None]
    return S


# ---------------------------------------------------------------------------
# Bass kernel: per-core out = clip(x + d, 0, 1) on a [128,147] f32 tile
# ---------------------------------------------------------------------------

_NC = None
_DELTA = None
LAST_EXEC_NS = None
PROFILE = False


def _build_nc():
    import concourse.bass as bass
    import concourse.mybir as mybir

    f32 = mybir.dt.float32
    nc = bass.Bass("TRN2", num_devices=N_CORES, use_seq_codegen=True)
    x_d = nc.declare_dram_parameter("x", [P, F], f32, isOutput=False)
    d_d = nc.declare_dram_parameter("d", [P, F], f32, isOutput=False)
    o_d = nc.declare_dram_parameter("out", [P, F], f32, isOutput=True)
    xt = nc.alloc_sbuf_tensor("xt_sb", [P, F], f32)
    dt = nc.alloc_sbuf_tensor("dt_sb", [P, F], f32)

    with (
        nc.Block(no_gpsimd_drain=True) as block,
        nc.semaphore("dma_sem") as dma_sem,
        nc.semaphore("v_sem") as v_sem,
    ):
        # x on the SP HWDGE ring, d on the ACT ring: the loads run in parallel.
        # No explicit wait on the out-DMA: the block-end DRAIN on SP flushes
        # its HWDGE ring before the engines halt (verified bit-exact).
        @block.sync
        def _(sync):
            sync.dma_start(out=xt[:], in_=x_d[:]).then_inc(dma_sem, 16)
            sync.wait_ge(v_sem, 1)
            sync.dma_start(out=o_d[:], in_=xt[:]).then_inc(dma_sem, 16)

        @block.scalar
        def _(scalar):
            scalar.dma_start(out=dt[:], in_=d_d[:]).then_inc(dma_sem, 16)

        @block.vector
        def _(vector):
            vector.wait_ge(dma_sem, 32)
            vector.tensor_add(out=xt[:], in0=xt[:], in1=dt[:])
            vector.tensor_scalar(
                out=xt[:],
                in0=xt[:],
                scalar1=0.0,
                scalar2=1.0,
                op0=mybir.AluOpType.max,
                op1=mybir.AluOpType.min,
            ).then_inc(v_sem, 1)

    return nc


def kernel(x):
    global _NC, _DELTA, LAST_EXEC_NS
    from concourse.bass_utils import run_bass_kernel_spmd

    if _NC is None:
        _NC = _build_nc()
    if _DELTA is None:
        _DELTA = (EPS32 * _sign_map()).astype(np.float32).reshape(N_CORES, P, F)

    x_np = np.asarray(x)
    in_dtype = x_np.dtype
    x_np = np.ascontiguousarray(x_np, dtype=np.float32).reshape(N_CORES, P, F)
    in_maps = [
        {"x": np.ascontiguousarray(x_np[i]), "d": _DELTA[i]} for i in range(N_CORES)
    ]
    res = run_bass_kernel_spmd(
        _NC, in_maps, core_ids=list(range(N_CORES)), trace=PROFILE
    )
    LAST_EXEC_NS = res.exec_time_ns
    out = np.stack([res.results[i]["out"] for i in range(N_CORES)])
    return out.reshape(1, C, H, H).astype(in_dtype)


# revision 7
# speedup vs baseline: 1.3528x; 1.3413x over previous
"""Trainium2 kernel for nn_Add_Square_2654289789550 (Square-Attack patch loop).

Key reduction: each of the 5000 steps adds +/-2*EPS to a square patch and then
clamps every pixel to [max(x-EPS,0), min(x+EPS,1)] -- an interval whose width
is <= 2*EPS.  Since the step magnitude (2*EPS) always covers the interval, a
touched pixel saturates to exactly `lo` or `hi` depending only on the sign of
the LAST patch that touched it; the init also places every pixel exactly at
lo/hi.  All randomness comes from the fixed key jax.random.key(1) -- fully
input-independent -- so the per-pixel final sign map S is a compile-time
constant (computed host-side with a bit-exact numpy port of jax's threefry
PRNG).  The device kernel then computes

    out = clip(x + EPS * S, 0, 1)

which matches the reference to <=2 ulp (verified: rel err 1.4e-8 against a
full f32 simulation of the 5000-step loop).

Distribution: x is flattened (150528 elems = 8 * 128 * 147) and split
contiguously across the 8 NeuronCores.  Per core (raw bass, no Tile):
x-load on the SP HWDGE ring and delta-load on the ACT ring in parallel,
DVE does one tensor_add + one fused two-scalar clamp, out-store on SP.
"""

import sys

import numpy as np

sys.path.insert(0, "/opt/trn_rl_repo")

N_CORES = 8
C, H = 3, 224
EPS32 = np.float32(0.05)
N_QUERIES = 5000
P_INIT = 0.8
P, F = 128, 147  # per-core tile: 128 partitions x 147 f32


# ---------------------------------------------------------------------------
# Pure-numpy, bit-exact port of jax.random's threefry2x32 PRNG
# ---------------------------------------------------------------------------

_ROT = ((13, 15, 26, 6), (17, 29, 16, 24))


def _rotl(x, r):
    return (x << np.uint32(r)) | (x >> np.uint32(32 - r))


def _threefry2x32(key, x0, x1):
    with np.errstate(over="ignore"):
        ks0 = np.asarray(key[0], np.uint32)
        ks1 = np.asarray(key[1], np.uint32)
        ks2 = ks0 ^ ks1 ^ np.uint32(0x1BD11BDA)
        x0 = x0.astype(np.uint32) + ks0
        x1 = x1.astype(np.uint32) + ks1
        ks = (ks0, ks1, ks2)
        for i in range(5):
            for r in _ROT[i % 2]:
                x0 = x0 + x1
                x1 = _rotl(x1, r)
                x1 = x1 ^ x0
            x0 = x0 + ks[(i + 1) % 3]
            x1 = x1 + ks[(i + 2) % 3] + np.uint32(i + 1)
        return x0, x1


def _threefry_counts(key, counts):
    counts = counts.ravel().astype(np.uint32)
    n = counts.size
    if n % 2:
        counts = np.concatenate([counts, np.zeros(1, np.uint32)])
    half = counts.size // 2
    y0, y1 = _threefry2x32(key, counts[:half], counts[half:])
    return np.concatenate([y0, y1])[:n]


def _fold_in(key, data):
    hi = np.uint32((int(data) >> 32) & 0xFFFFFFFF)
    lo = np.uint32(int(data) & 0xFFFFFFFF)
    y0, y1 = _threefry2x32((key[0], key[1]), np.array([hi]), np.array([lo]))
    return np.array([y0[0], y1[0]], dtype=np.uint32)


def _split(key):
    out = _threefry_counts((key[0], key[1]), np.arange(4, dtype=np.uint32))
    return out[:2], out[2:]


def _uniform(key, n):
    bits = _threefry_counts((key[0], key[1]), np.arange(n, dtype=np.uint32))
    return _bits_to_uniform(bits)


def _bits_to_uniform(bits):
    f = ((bits >> np.uint32(9)) | np.uint32(0x3F800000)).view(np.float32)
    return np.maximum(np.float32(0.0), f - np.float32(1.0))


# ---------------------------------------------------------------------------
# Reference schedule + final sign map (input-independent constant)
# ---------------------------------------------------------------------------


def _p_selection(it):
    if 10 < it <= 50:
        return P_INIT / 2
    elif 50 < it <= 200:
        return P_INIT / 4
    elif 200 < it <= 500:
        return P_INIT / 8
    elif 500 < it <= 1000:
        return P_INIT / 16
    elif 1000 < it <= 2000:
        return P_INIT / 32
    elif 2000 < it <= 4000:
        return P_INIT / 64
    elif 4000 < it <= 6000:
        return P_INIT / 128
    elif 6000 < it <= 8000:
        return P_INIT / 256
    elif 8000 < it:
        return P_INIT / 512
    return P_INIT


def _s_schedule():
    import math

    n_features = C * H * H
    return np.array(
        [
            max(int(round(math.sqrt(_p_selection(i) * n_features / C))), 1)
            for i in range(N_QUERIES)
        ],
        dtype=np.int32,
    )


def _sign_map():
    N = N_QUERIES
    base = np.array([0, 1], dtype=np.uint32)  # jax.random.key(1)
    k0 = _fold_in(base, 0)
    init_sign = np.sign(
        np.float32(2.0) * _uniform(k0, C * H) - np.float32(1.0)
    ).reshape(C, 1, H)

    # batched fold_in(base, t+1): cipher(key=base, (0, t+1))
    ts = np.arange(1, N + 1, dtype=np.uint32)
    f0, f1 = _threefry2x32((base[0], base[1]), np.zeros(N, np.uint32), ts)
    # batched split: counts [0,1,2,3] -> pairs (0,2) and (1,3);
    # k1 = (y0 of both pairs), k2 = (y1 of both pairs)
    y0a, y1a = _threefry2x32((f0, f1), np.zeros(N, np.uint32), np.full(N, 2, np.uint32))
    y0b, y1b = _threefry2x32((f0, f1), np.ones(N, np.uint32), np.full(N, 3, np.uint32))
    # u = uniform(k1, ()): counts [0] padded to (0,0)
    ub, _ = _threefry2x32((y0a, y0b), np.zeros(N, np.uint32), np.zeros(N, np.uint32))
    us = _bits_to_uniform(ub)
    # sgn = uniform(k2, (3,)): counts [0,1,2] padded to pairs (0,2),(1,0)
    sa0, sa1 = _threefry2x32((y1a, y1b), np.zeros(N, np.uint32), np.full(N, 2, np.uint32))
    sb0, _ = _threefry2x32((y1a, y1b), np.ones(N, np.uint32), np.zeros(N, np.uint32))
    sg = np.sign(
        np.float32(2.0) * _bits_to_uniform(np.stack([sa0, sb0, sa1], axis=1))
        - np.float32(1.0)
    )

    s_arr = _s_schedule()
    vh = np.floor(us * (np.float32(H) - s_arr.astype(np.float32))).astype(np.int32)
    S = np.broadcast_to(init_sign, (C, H, H)).astype(np.float32).copy()
    for t in range(N):
        v, s = int(vh[t]), int(s_arr[t])
        S[:, v : v + s, v : v + s] = sg[t][:, None, None]
    return S


# ---------------------------------------------------------------------------
# Bass kernel: per-core out = clip(x + d, 0, 1) on a [128,147] f32 tile
# ---------------------------------------------------------------------------

_NC = None
_DELTA = None
LAST_EXEC_NS = None
PROFILE = False


def _build_nc():
    import concourse.bass as bass
    import concourse.mybir as mybir

    f32 = mybir.dt.float32
    nc = bass.Bass("TRN2", num_devices=N_CORES, use_seq_codegen=True)
    x_d = nc.declare_dram_parameter("x", [P, F], f32, isOutput=False)
    d_d = nc.declare_dram_parameter("d", [P, F], f32, isOutput=False)
    o_d = nc.declare_dram_parameter("out", [P, F], f32, isOutput=True)
    xt = nc.alloc_sbuf_tensor("xt_sb", [P, F], f32)
    dt = nc.alloc_sbuf_tensor("dt_sb", [P, F], f32)
    dma_sem = nc.ctx.enter_context(nc.semaphore("dma_sem"))
    v_sem = nc.ctx.enter_context(nc.semaphore("v_sem"))

    # Input DMAs emitted into the main BB (x on the SP HWDGE ring, d on the
    # ACT ring, running in parallel); the surgery below then hoists them to
    # the very front of the BB so the transfers overlap the framework
    # preamble (TENSOR_LOADs, register init) instead of running after it.
    nc.sync.dma_start(out=xt[:], in_=x_d[:]).then_inc(dma_sem, 16)
    nc.scalar.dma_start(out=dt[:], in_=d_d[:]).then_inc(dma_sem, 16)

    with nc.Block(no_gpsimd_drain=True) as block:
        # No explicit wait on the out-DMA: the block-end DRAIN on SP flushes
        # its HWDGE ring before the engines halt (verified bit-exact).
        @block.sync
        def _(sync):
            sync.wait_ge(v_sem, 1)
            sync.dma_start(out=o_d[:], in_=xt[:]).then_inc(dma_sem, 16)

        @block.vector
        def _(vector):
            vector.wait_ge(dma_sem, 32)
            vector.tensor_add(out=xt[:], in0=xt[:], in1=dt[:])
            vector.tensor_scalar(
                out=xt[:],
                in0=xt[:],
                scalar1=0.0,
                scalar2=1.0,
                op0=mybir.AluOpType.max,
                op1=mybir.AluOpType.min,
            ).then_inc(v_sem, 1)

    # --- BIR surgery on the main BB ---
    # 1. Hoist the two input InstDMACopy to the front (right after the
    #    dummycall) so descriptor generation + the transfers run during the
    #    fixed preamble phases.
    # 2. Drop the framework's const-tile InstMemsets: walrus itself warns
    #    they have no reader in this kernel (our clamp scalars are
    #    immediates), and their presence stalls the preamble by ~2us.
    main = nc.m.functions[0].blocks[0]
    insts = main.instructions
    dmas = [i for i, ins in enumerate(insts) if type(ins).__name__ == "InstDMACopy"]
    assert len(dmas) == 2, f"expected the 2 input DMAs in main BB, got {dmas}"
    moved = [insts[i] for i in dmas]
    for i in reversed(dmas):
        del insts[i]
    for j, ins in enumerate(moved):
        insts.insert(1 + j, ins)
    memsets = [i for i, ins in enumerate(insts) if type(ins).__name__ == "InstMemset"]
    for i in reversed(memsets):
        del insts[i]

    return nc


def kernel(x):
    global _NC, _DELTA, LAST_EXEC_NS
    from concourse.bass_utils import run_bass_kernel_spmd

    if _NC is None:
        _NC = _build_nc()
    if _DELTA is None:
        _DELTA = (EPS32 * _sign_map()).astype(np.float32).reshape(N_CORES, P, F)

    x_np = np.asarray(x)
    in_dtype = x_np.dtype
    x_np = np.ascontiguousarray(x_np, dtype=np.float32).reshape(N_CORES, P, F)
    in_maps = [
        {"x": np.ascontiguousarray(x_np[i]), "d": _DELTA[i]} for i in range(N_CORES)
    ]
    res = run_bass_kernel_spmd(
        _NC, in_maps, core_ids=list(range(N_CORES)), trace=PROFILE
    )
    LAST_EXEC_NS = res.exec_time_ns
    out = np.stack([res.results[i]["out"] for i in range(N_CORES)])
    return out.reshape(1, C, H, H).astype(in_dtype)


# revision 8
# speedup vs baseline: 1.4325x; 1.0590x over previous
"""Trainium2 kernel for nn_Add_Square_2654289789550 (Square-Attack patch loop).

Key reduction: each of the 5000 steps adds +/-2*EPS to a square patch and then
clamps every pixel to [max(x-EPS,0), min(x+EPS,1)] -- an interval whose width
is <= 2*EPS.  Since the step magnitude (2*EPS) always covers the interval, a
touched pixel saturates to exactly `lo` or `hi` depending only on the sign of
the LAST patch that touched it; the init also places every pixel exactly at
lo/hi.  All randomness comes from the fixed key jax.random.key(1) -- fully
input-independent -- so the per-pixel final sign map S is a compile-time
constant (computed host-side with a bit-exact numpy port of jax's threefry
PRNG).  The device kernel then computes

    out = clip(x + EPS * S, 0, 1)

which matches the reference to <=2 ulp (verified: rel err 1.4e-8 against a
full f32 simulation of the 5000-step loop).

Distribution: x is flattened (150528 elems = 8 * 128 * 147) and split
contiguously across the 8 NeuronCores.  Per core (raw bass, no Tile):
x-load on the SP HWDGE ring and delta-load on the ACT ring in parallel,
DVE does one tensor_add + one fused two-scalar clamp, out-store on SP.
"""

import sys

import numpy as np

sys.path.insert(0, "/opt/trn_rl_repo")

N_CORES = 8
C, H = 3, 224
EPS32 = np.float32(0.05)
N_QUERIES = 5000
P_INIT = 0.8
P, F = 128, 147  # per-core tile: 128 partitions x 147 f32


# ---------------------------------------------------------------------------
# Pure-numpy, bit-exact port of jax.random's threefry2x32 PRNG
# ---------------------------------------------------------------------------

_ROT = ((13, 15, 26, 6), (17, 29, 16, 24))


def _rotl(x, r):
    return (x << np.uint32(r)) | (x >> np.uint32(32 - r))


def _threefry2x32(key, x0, x1):
    with np.errstate(over="ignore"):
        ks0 = np.asarray(key[0], np.uint32)
        ks1 = np.asarray(key[1], np.uint32)
        ks2 = ks0 ^ ks1 ^ np.uint32(0x1BD11BDA)
        x0 = x0.astype(np.uint32) + ks0
        x1 = x1.astype(np.uint32) + ks1
        ks = (ks0, ks1, ks2)
        for i in range(5):
            for r in _ROT[i % 2]:
                x0 = x0 + x1
                x1 = _rotl(x1, r)
                x1 = x1 ^ x0
            x0 = x0 + ks[(i + 1) % 3]
            x1 = x1 + ks[(i + 2) % 3] + np.uint32(i + 1)
        return x0, x1


def _threefry_counts(key, counts):
    counts = counts.ravel().astype(np.uint32)
    n = counts.size
    if n % 2:
        counts = np.concatenate([counts, np.zeros(1, np.uint32)])
    half = counts.size // 2
    y0, y1 = _threefry2x32(key, counts[:half], counts[half:])
    return np.concatenate([y0, y1])[:n]


def _fold_in(key, data):
    hi = np.uint32((int(data) >> 32) & 0xFFFFFFFF)
    lo = np.uint32(int(data) & 0xFFFFFFFF)
    y0, y1 = _threefry2x32((key[0], key[1]), np.array([hi]), np.array([lo]))
    return np.array([y0[0], y1[0]], dtype=np.uint32)


def _split(key):
    out = _threefry_counts((key[0], key[1]), np.arange(4, dtype=np.uint32))
    return out[:2], out[2:]


def _uniform(key, n):
    bits = _threefry_counts((key[0], key[1]), np.arange(n, dtype=np.uint32))
    return _bits_to_uniform(bits)


def _bits_to_uniform(bits):
    f = ((bits >> np.uint32(9)) | np.uint32(0x3F800000)).view(np.float32)
    return np.maximum(np.float32(0.0), f - np.float32(1.0))


# ---------------------------------------------------------------------------
# Reference schedule + final sign map (input-independent constant)
# ---------------------------------------------------------------------------


def _p_selection(it):
    if 10 < it <= 50:
        return P_INIT / 2
    elif 50 < it <= 200:
        return P_INIT / 4
    elif 200 < it <= 500:
        return P_INIT / 8
    elif 500 < it <= 1000:
        return P_INIT / 16
    elif 1000 < it <= 2000:
        return P_INIT / 32
    elif 2000 < it <= 4000:
        return P_INIT / 64
    elif 4000 < it <= 6000:
        return P_INIT / 128
    elif 6000 < it <= 8000:
        return P_INIT / 256
    elif 8000 < it:
        return P_INIT / 512
    return P_INIT


def _s_schedule():
    import math

    n_features = C * H * H
    return np.array(
        [
            max(int(round(math.sqrt(_p_selection(i) * n_features / C))), 1)
            for i in range(N_QUERIES)
        ],
        dtype=np.int32,
    )


def _sign_map():
    N = N_QUERIES
    base = np.array([0, 1], dtype=np.uint32)  # jax.random.key(1)
    k0 = _fold_in(base, 0)
    init_sign = np.sign(
        np.float32(2.0) * _uniform(k0, C * H) - np.float32(1.0)
    ).reshape(C, 1, H)

    # batched fold_in(base, t+1): cipher(key=base, (0, t+1))
    ts = np.arange(1, N + 1, dtype=np.uint32)
    f0, f1 = _threefry2x32((base[0], base[1]), np.zeros(N, np.uint32), ts)
    # batched split: counts [0,1,2,3] -> pairs (0,2) and (1,3);
    # k1 = (y0 of both pairs), k2 = (y1 of both pairs)
    y0a, y1a = _threefry2x32((f0, f1), np.zeros(N, np.uint32), np.full(N, 2, np.uint32))
    y0b, y1b = _threefry2x32((f0, f1), np.ones(N, np.uint32), np.full(N, 3, np.uint32))
    # u = uniform(k1, ()): counts [0] padded to (0,0)
    ub, _ = _threefry2x32((y0a, y0b), np.zeros(N, np.uint32), np.zeros(N, np.uint32))
    us = _bits_to_uniform(ub)
    # sgn = uniform(k2, (3,)): counts [0,1,2] padded to pairs (0,2),(1,0)
    sa0, sa1 = _threefry2x32((y1a, y1b), np.zeros(N, np.uint32), np.full(N, 2, np.uint32))
    sb0, _ = _threefry2x32((y1a, y1b), np.ones(N, np.uint32), np.zeros(N, np.uint32))
    sg = np.sign(
        np.float32(2.0) * _bits_to_uniform(np.stack([sa0, sb0, sa1], axis=1))
        - np.float32(1.0)
    )

    s_arr = _s_schedule()
    vh = np.floor(us * (np.float32(H) - s_arr.astype(np.float32))).astype(np.int32)
    S = np.broadcast_to(init_sign, (C, H, H)).astype(np.float32).copy()
    for t in range(N):
        v, s = int(vh[t]), int(s_arr[t])
        S[:, v : v + s, v : v + s] = sg[t][:, None, None]
    return S


# ---------------------------------------------------------------------------
# Bass kernel: per-core out = clip(x + d, 0, 1) on a [128,147] f32 tile
# ---------------------------------------------------------------------------

_NC = None
_DELTA = None
LAST_EXEC_NS = None
PROFILE = False


def _build_nc():
    import concourse.bass as bass
    import concourse.mybir as mybir

    f32 = mybir.dt.float32
    nc = bass.Bass("TRN2", num_devices=N_CORES, use_seq_codegen=True)
    x_d = nc.declare_dram_parameter("x", [P, F], f32, isOutput=False)
    d_d = nc.declare_dram_parameter("d", [P, F], f32, isOutput=False)
    o_d = nc.declare_dram_parameter("out", [P, F], f32, isOutput=True)
    xt = nc.alloc_sbuf_tensor("xt_sb", [P, F], f32)
    dt = nc.alloc_sbuf_tensor("dt_sb", [P, F], f32)
    dma_sem = nc.ctx.enter_context(nc.semaphore("dma_sem"))
    v_sem = nc.ctx.enter_context(nc.semaphore("v_sem"))

    # Input DMAs emitted into the main BB (x on the SP HWDGE ring, d on the
    # ACT ring, running in parallel); the surgery below then hoists them to
    # the very front of the BB so the transfers overlap the framework
    # preamble (TENSOR_LOADs, register init) instead of running after it.
    nc.sync.dma_start(out=xt[:], in_=x_d[:]).then_inc(dma_sem, 16)
    nc.scalar.dma_start(out=dt[:], in_=d_d[:]).then_inc(dma_sem, 16)

    with nc.Block(no_gpsimd_drain=True) as block:
        # No explicit wait on the out-DMA: the block-end DRAIN on SP flushes
        # its HWDGE ring before the engines halt (verified bit-exact).
        @block.sync
        def _(sync):
            sync.wait_ge(v_sem, 1)
            sync.dma_start(out=o_d[:], in_=xt[:]).then_inc(dma_sem, 16)

        @block.vector
        def _(vector):
            vector.wait_ge(dma_sem, 32)
            vector.tensor_add(out=xt[:], in0=xt[:], in1=dt[:])
            vector.tensor_scalar(
                out=xt[:],
                in0=xt[:],
                scalar1=0.0,
                scalar2=1.0,
                op0=mybir.AluOpType.max,
                op1=mybir.AluOpType.min,
            ).then_inc(v_sem, 1)

    # --- BIR surgery ---
    # 1. Hoist the two input InstDMACopy to the front of the main BB (right
    #    after the dummycall) so descriptor generation + the transfers run
    #    during the fixed preamble phases.
    # 2. Drop the framework's const-tile InstMemsets: walrus itself warns
    #    they have no reader in this kernel (our clamp scalars are
    #    immediates), and their presence stalls the preamble by ~2us.
    # 3. Drop the block-end aeb_* barrier EventSemaphores but KEEP the
    #    per-engine InstDrains -- the SP drain is what guarantees the
    #    output DMA completed before the engines halt.
    main = nc.m.functions[0].blocks[0]
    insts = main.instructions
    dmas = [i for i, ins in enumerate(insts) if type(ins).__name__ == "InstDMACopy"]
    assert len(dmas) == 2, f"expected the 2 input DMAs in main BB, got {dmas}"
    moved = [insts[i] for i in dmas]
    for i in reversed(dmas):
        del insts[i]
    for j, ins in enumerate(moved):
        insts.insert(1 + j, ins)
    memsets = [i for i, ins in enumerate(insts) if type(ins).__name__ == "InstMemset"]
    for i in reversed(memsets):
        del insts[i]
    for bb in nc.m.functions[0].blocks:
        if bb.name.endswith("_end"):
            kept = [
                ins
                for ins in bb.instructions
                if not (
                    type(ins).__name__ == "InstEventSemaphore"
                    and ins.name.startswith("aeb_")
                )
            ]
            assert any(type(i).__name__ == "InstDrain" for i in kept), (
                "block-end drains must survive the barrier strip"
            )
            bb.instructions[:] = kept

    return nc


def kernel(x):
    global _NC, _DELTA, LAST_EXEC_NS
    from concourse.bass_utils import run_bass_kernel_spmd

    if _NC is None:
        _NC = _build_nc()
    if _DELTA is None:
        _DELTA = (EPS32 * _sign_map()).astype(np.float32).reshape(N_CORES, P, F)

    x_np = np.asarray(x)
    in_dtype = x_np.dtype
    x_np = np.ascontiguousarray(x_np, dtype=np.float32).reshape(N_CORES, P, F)
    in_maps = [
        {"x": np.ascontiguousarray(x_np[i]), "d": _DELTA[i]} for i in range(N_CORES)
    ]
    res = run_bass_kernel_spmd(
        _NC, in_maps, core_ids=list(range(N_CORES)), trace=PROFILE
    )
    LAST_EXEC_NS = res.exec_time_ns
    out = np.stack([res.results[i]["out"] for i in range(N_CORES)])
    return out.reshape(1, C, H, H).astype(in_dtype)


# revision 9
# speedup vs baseline: 1.4327x; 1.0001x over previous
"""Trainium2 kernel for nn_Add_Square_2654289789550 (Square-Attack patch loop).

Key reduction: each of the 5000 steps adds +/-2*EPS to a square patch and then
clamps every pixel to [max(x-EPS,0), min(x+EPS,1)] -- an interval whose width
is <= 2*EPS.  Since the step magnitude (2*EPS) always covers the interval, a
touched pixel saturates to exactly `lo` or `hi` depending only on the sign of
the LAST patch that touched it; the init also places every pixel exactly at
lo/hi.  All randomness comes from the fixed key jax.random.key(1) -- fully
input-independent -- so the per-pixel final sign map S is a compile-time
constant (computed host-side with a bit-exact numpy port of jax's threefry
PRNG).  The device kernel then computes

    out = clip(x + EPS * S, 0, 1)

which matches the reference to <=2 ulp (verified: rel err 1.4e-8 against a
full f32 simulation of the 5000-step loop).

Distribution: x is flattened (150528 elems = 8 * 128 * 147) and split
contiguously across the 8 NeuronCores.  Per core (raw bass, no Tile):
x-load on the SP HWDGE ring and delta-load on the ACT ring in parallel,
DVE does one tensor_add + one fused two-scalar clamp, out-store on SP.
"""

import sys

import numpy as np

sys.path.insert(0, "/opt/trn_rl_repo")

N_CORES = 8
C, H = 3, 224
EPS32 = np.float32(0.05)
N_QUERIES = 5000
P_INIT = 0.8
P, F = 128, 147  # per-core tile: 128 partitions x 147 f32


# ---------------------------------------------------------------------------
# Pure-numpy, bit-exact port of jax.random's threefry2x32 PRNG
# ---------------------------------------------------------------------------

_ROT = ((13, 15, 26, 6), (17, 29, 16, 24))


def _rotl(x, r):
    return (x << np.uint32(r)) | (x >> np.uint32(32 - r))


def _threefry2x32(key, x0, x1):
    with np.errstate(over="ignore"):
        ks0 = np.asarray(key[0], np.uint32)
        ks1 = np.asarray(key[1], np.uint32)
        ks2 = ks0 ^ ks1 ^ np.uint32(0x1BD11BDA)
        x0 = x0.astype(np.uint32) + ks0
        x1 = x1.astype(np.uint32) + ks1
        ks = (ks0, ks1, ks2)
        for i in range(5):
            for r in _ROT[i % 2]:
                x0 = x0 + x1
                x1 = _rotl(x1, r)
                x1 = x1 ^ x0
            x0 = x0 + ks[(i + 1) % 3]
            x1 = x1 + ks[(i + 2) % 3] + np.uint32(i + 1)
        return x0, x1


def _threefry_counts(key, counts):
    counts = counts.ravel().astype(np.uint32)
    n = counts.size
    if n % 2:
        counts = np.concatenate([counts, np.zeros(1, np.uint32)])
    half = counts.size // 2
    y0, y1 = _threefry2x32(key, counts[:half], counts[half:])
    return np.concatenate([y0, y1])[:n]


def _fold_in(key, data):
    hi = np.uint32((int(data) >> 32) & 0xFFFFFFFF)
    lo = np.uint32(int(data) & 0xFFFFFFFF)
    y0, y1 = _threefry2x32((key[0], key[1]), np.array([hi]), np.array([lo]))
    return np.array([y0[0], y1[0]], dtype=np.uint32)


def _split(key):
    out = _threefry_counts((key[0], key[1]), np.arange(4, dtype=np.uint32))
    return out[:2], out[2:]


def _uniform(key, n):
    bits = _threefry_counts((key[0], key[1]), np.arange(n, dtype=np.uint32))
    return _bits_to_uniform(bits)


def _bits_to_uniform(bits):
    f = ((bits >> np.uint32(9)) | np.uint32(0x3F800000)).view(np.float32)
    return np.maximum(np.float32(0.0), f - np.float32(1.0))


# ---------------------------------------------------------------------------
# Reference schedule + final sign map (input-independent constant)
# ---------------------------------------------------------------------------


def _p_selection(it):
    if 10 < it <= 50:
        return P_INIT / 2
    elif 50 < it <= 200:
        return P_INIT / 4
    elif 200 < it <= 500:
        return P_INIT / 8
    elif 500 < it <= 1000:
        return P_INIT / 16
    elif 1000 < it <= 2000:
        return P_INIT / 32
    elif 2000 < it <= 4000:
        return P_INIT / 64
    elif 4000 < it <= 6000:
        return P_INIT / 128
    elif 6000 < it <= 8000:
        return P_INIT / 256
    elif 8000 < it:
        return P_INIT / 512
    return P_INIT


def _s_schedule():
    import math

    n_features = C * H * H
    return np.array(
        [
            max(int(round(math.sqrt(_p_selection(i) * n_features / C))), 1)
            for i in range(N_QUERIES)
        ],
        dtype=np.int32,
    )


def _sign_map():
    N = N_QUERIES
    base = np.array([0, 1], dtype=np.uint32)  # jax.random.key(1)
    k0 = _fold_in(base, 0)
    init_sign = np.sign(
        np.float32(2.0) * _uniform(k0, C * H) - np.float32(1.0)
    ).reshape(C, 1, H)

    # batched fold_in(base, t+1): cipher(key=base, (0, t+1))
    ts = np.arange(1, N + 1, dtype=np.uint32)
    f0, f1 = _threefry2x32((base[0], base[1]), np.zeros(N, np.uint32), ts)
    # batched split: counts [0,1,2,3] -> pairs (0,2) and (1,3);
    # k1 = (y0 of both pairs), k2 = (y1 of both pairs)
    y0a, y1a = _threefry2x32((f0, f1), np.zeros(N, np.uint32), np.full(N, 2, np.uint32))
    y0b, y1b = _threefry2x32((f0, f1), np.ones(N, np.uint32), np.full(N, 3, np.uint32))
    # u = uniform(k1, ()): counts [0] padded to (0,0)
    ub, _ = _threefry2x32((y0a, y0b), np.zeros(N, np.uint32), np.zeros(N, np.uint32))
    us = _bits_to_uniform(ub)
    # sgn = uniform(k2, (3,)): counts [0,1,2] padded to pairs (0,2),(1,0)
    sa0, sa1 = _threefry2x32((y1a, y1b), np.zeros(N, np.uint32), np.full(N, 2, np.uint32))
    sb0, _ = _threefry2x32((y1a, y1b), np.ones(N, np.uint32), np.zeros(N, np.uint32))
    sg = np.sign(
        np.float32(2.0) * _bits_to_uniform(np.stack([sa0, sb0, sa1], axis=1))
        - np.float32(1.0)
    )

    s_arr = _s_schedule()
    vh = np.floor(us * (np.float32(H) - s_arr.astype(np.float32))).astype(np.int32)
    S = np.broadcast_to(init_sign, (C, H, H)).astype(np.float32).copy()
    for t in range(N):
        v, s = int(vh[t]), int(s_arr[t])
        S[:, v : v + s, v : v + s] = sg[t][:, None, None]
    return S


# ---------------------------------------------------------------------------
# Bass kernel: per-core out = clip(x + d, 0, 1) on a [128,147] f32 tile
# ---------------------------------------------------------------------------

_NC = None
_DELTA = None
LAST_EXEC_NS = None
PROFILE = False


def _build_nc():
    import concourse.bass as bass
    import concourse.mybir as mybir

    f32 = mybir.dt.float32
    nc = bass.Bass("TRN2", num_devices=N_CORES, use_seq_codegen=True)
    x_d = nc.declare_dram_parameter("x", [P, F], f32, isOutput=False)
    d_d = nc.declare_dram_parameter("d", [P, F], f32, isOutput=False)
    o_d = nc.declare_dram_parameter("out", [P, F], f32, isOutput=True)
    xt = nc.alloc_sbuf_tensor("xt_sb", [P, F], f32)
    dt = nc.alloc_sbuf_tensor("dt_sb", [P, F], f32)
    dma_sem = nc.ctx.enter_context(nc.semaphore("dma_sem"))
    v_sem = nc.ctx.enter_context(nc.semaphore("v_sem"))

    # Input DMAs emitted into the main BB (x on the SP HWDGE ring, d on the
    # ACT ring, running in parallel); the surgery below then hoists them to
    # the very front of the BB so the transfers overlap the framework
    # preamble (TENSOR_LOADs, register init) instead of running after it.
    nc.sync.dma_start(out=xt[:], in_=x_d[:], single_packet=True).then_inc(
        dma_sem, 16
    )
    nc.scalar.dma_start(out=dt[:], in_=d_d[:], single_packet=True).then_inc(
        dma_sem, 16
    )

    with nc.Block(no_gpsimd_drain=True) as block:
        # No explicit wait on the out-DMA: the block-end DRAIN on SP flushes
        # its HWDGE ring before the engines halt (verified bit-exact).
        @block.sync
        def _(sync):
            sync.wait_ge(v_sem, 1)
            sync.dma_start(out=o_d[:], in_=xt[:], single_packet=True).then_inc(
                dma_sem, 16
            )

        @block.vector
        def _(vector):
            vector.wait_ge(dma_sem, 32)
            vector.tensor_add(out=xt[:], in0=xt[:], in1=dt[:])
            vector.tensor_scalar(
                out=xt[:],
                in0=xt[:],
                scalar1=0.0,
                scalar2=1.0,
                op0=mybir.AluOpType.max,
                op1=mybir.AluOpType.min,
            ).then_inc(v_sem, 1)

    # --- BIR surgery ---
    # 1. Hoist the two input InstDMACopy to the front of the main BB (right
    #    after the dummycall) so descriptor generation + the transfers run
    #    during the fixed preamble phases.
    # 2. Drop the framework's const-tile InstMemsets: walrus itself warns
    #    they have no reader in this kernel (our clamp scalars are
    #    immediates), and their presence stalls the preamble by ~2us.
    # 3. Drop the block-end aeb_* barrier EventSemaphores but KEEP the
    #    per-engine InstDrains -- the SP drain is what guarantees the
    #    output DMA completed before the engines halt.
    main = nc.m.functions[0].blocks[0]
    insts = main.instructions
    dmas = [i for i, ins in enumerate(insts) if type(ins).__name__ == "InstDMACopy"]
    assert len(dmas) == 2, f"expected the 2 input DMAs in main BB, got {dmas}"
    moved = [insts[i] for i in dmas]
    for i in reversed(dmas):
        del insts[i]
    for j, ins in enumerate(moved):
        insts.insert(1 + j, ins)
    memsets = [i for i, ins in enumerate(insts) if type(ins).__name__ == "InstMemset"]
    for i in reversed(memsets):
        del insts[i]
    for bb in nc.m.functions[0].blocks:
        if bb.name.endswith("_end"):
            kept = [
                ins
                for ins in bb.instructions
                if not (
                    type(ins).__name__ == "InstEventSemaphore"
                    and ins.name.startswith("aeb_")
                )
            ]
            assert any(type(i).__name__ == "InstDrain" for i in kept), (
                "block-end drains must survive the barrier strip"
            )
            bb.instructions[:] = kept

    return nc


def kernel(x):
    global _NC, _DELTA, LAST_EXEC_NS
    from concourse.bass_utils import run_bass_kernel_spmd

    if _NC is None:
        _NC = _build_nc()
    if _DELTA is None:
        _DELTA = (EPS32 * _sign_map()).astype(np.float32).reshape(N_CORES, P, F)

    x_np = np.asarray(x)
    in_dtype = x_np.dtype
    x_np = np.ascontiguousarray(x_np, dtype=np.float32).reshape(N_CORES, P, F)
    in_maps = [
        {"x": np.ascontiguousarray(x_np[i]), "d": _DELTA[i]} for i in range(N_CORES)
    ]
    res = run_bass_kernel_spmd(
        _NC, in_maps, core_ids=list(range(N_CORES)), trace=PROFILE
    )
    LAST_EXEC_NS = res.exec_time_ns
    out = np.stack([res.results[i]["out"] for i in range(N_CORES)])
    return out.reshape(1, C, H, H).astype(in_dtype)
